# revision 1
# baseline (speedup 1.0000x reference)
"""Multihead causal attention on 8 TRN2 NeuronCores.

Sharding: core = (batch b, head-group hg): b = core//2, hg = core%2.
Each core gets x[b] (full sequence, [2048, 1024]) plus the weight rows for
its 8 heads (W[hg*512:(hg+1)*512, :]), computes Q/K/V projections and
causal attention for those (batch, head) pairs, and writes Y transposed
as [8, 64, 2048] (head, dh, seq); the host transposes back on gather.

On-device dataflow (per core):
  - x.T and W.T tiles built with PE transposes (contraction dim must sit
    on SBUF partitions).
  - Q.T, K.T in [d, s] layout, V in [s, d] layout; matmuls run as
    float32r (1 cyc/row vs 4 for strict fp32).
  - Attention in transposed-score layout: scoresT[k, q] = K @ Q.T per
    head, two heads packed in the 128-row PE array (K=64 each).
  - Softmax without a max pass (scaled scores are O(4)); exp on ScalarE
    (PSUM -> bf16 SBUF), causal mask via zero-prefix memset + one
    [128,128] triangular 0/1 multiply per diagonal tile.
  - PV matmul in bf16 with a ones-column appended to V: out [65, 512]
    rows 0..63 = unnormalized Y.T, row 64 = softmax denominator.
  - Normalize: reciprocal of the denominator row, partition-broadcast
    via a DRAM bounce, one DVE multiply.
"""
import numpy as np
import ml_dtypes

import concourse.bass as bass
import concourse.tile as tile
from concourse import bacc, mybir
from concourse.bass_utils import run_bass_kernel_spmd

F32 = mybir.dt.float32
F32R = mybir.dt.float32r
BF16 = mybir.dt.bfloat16
EXP = mybir.ActivationFunctionType.Exp

B, S, D, H, DH = 4, 2048, 1024, 16, 64
N_CORES = 8
H_LOC = 8          # heads per core
D_LOC = H_LOC * DH  # 512: projection output dim per core
N_CT = D // 128     # 8 contraction tiles
N_ST = S // 128     # 16 sequence tiles of 128
N_QT = S // 512     # 4 q-chunks of 512
SCALE = 1.0 / np.sqrt(DH)

_NC_CACHE = {}


def build_nc():
    nc = bacc.Bacc("TRN2", target_bir_lowering=False, debug=False,
                   num_devices=N_CORES)
    xtd = nc.dram_tensor("xtd", [D, S], BF16, kind="ExternalInput").ap()
    wqt = nc.dram_tensor("wqt", [D, D_LOC], BF16, kind="ExternalInput").ap()
    wkt = nc.dram_tensor("wkt", [D, D_LOC], BF16, kind="ExternalInput").ap()
    wvt = nc.dram_tensor("wvt", [D, D_LOC], BF16, kind="ExternalInput").ap()
    out = nc.dram_tensor("out", [H_LOC, DH, S], F32, kind="ExternalOutput").ap()

    # tri[kk, qq] = 1 iff qq >= kk (valid: query position >= key position)
    tri_np = (np.arange(128)[None, :] >= np.arange(128)[:, None])
    tri_dram = nc.inline_tensor(tri_np.astype(ml_dtypes.bfloat16), name="tri")

    with tile.TileContext(nc) as tc:
        with tc.tile_pool(name="consts", bufs=1) as consts, \
             tc.tile_pool(name="pers", bufs=1) as pers:
            tri = consts.tile([128, 128], BF16)
            nc.sync.dma_start(tri[:], tri_dram.ap())

            # persistent per-core tensors
            QT = [pers.tile([128, S], BF16, tag=f"QT{i}", name=f"QT{i}") for i in range(4)]
            KT = [pers.tile([128, S], BF16, tag=f"KT{i}", name=f"KT{i}") for i in range(4)]
            VP = [pers.tile([128, H_LOC, DH + 1], BF16, tag=f"VP{i}", name=f"VP{i}")
                  for i in range(N_ST)]

            # ---- phase A+B: DMA-transposes + projections (all bf16) ----
            with tc.tile_pool(name="xT", bufs=1) as xT_pool, \
                 tc.tile_pool(name="WT", bufs=1) as WT_pool, \
                 tc.tile_pool(name="psP", bufs=2, space="PSUM") as psP:

                # x.T supplied pre-transposed by the host: plain loads
                xT = [xT_pool.tile([128, S], BF16, tag=f"xT{i}", name=f"xT{i}")
                      for i in range(N_CT)]
                for ct in range(N_CT):
                    nc.sync.dma_start(
                        xT[ct][:], xtd[ct * 128:(ct + 1) * 128, :])

                WT = [WT_pool.tile([128, D_LOC], BF16, tag=f"WT{i}",
                                   name=f"WT{i}") for i in range(N_CT)]
                for wi, w in ((2, wvt), (1, wkt), (0, wqt)):
                    for ct in range(N_CT):
                        nc.sync.dma_start(
                            WT[ct][:], w[ct * 128:(ct + 1) * 128, :])
                    if wi < 2:  # Q or K: out[dloc, s]
                        dst = QT if wi == 0 else KT
                        for dt in range(4):
                            for qc in range(N_QT):
                                pp = psP.tile([128, 512], F32, tag="pp",
                                              name="pp")
                                for ct in range(N_CT):
                                    nc.tensor.matmul(
                                        pp[:],
                                        WT[ct][:, dt * 128:(dt + 1) * 128],
                                        xT[ct][:, qc * 512:(qc + 1) * 512],
                                        start=(ct == 0), stop=(ct == N_CT - 1))
                                nc.vector.tensor_copy(
                                    dst[dt][:, qc * 512:(qc + 1) * 512], pp[:])
                    else:  # V: out[s, dloc] -> VP interleaved by head
                        for st in range(N_ST):
                            pp = psP.tile([128, 512], F32, tag="pp", name="pp")
                            for ct in range(N_CT):
                                nc.tensor.matmul(
                                    pp[:],
                                    xT[ct][:, st * 128:(st + 1) * 128],
                                    WT[ct][:],
                                    start=(ct == 0), stop=(ct == N_CT - 1))
                            nc.vector.tensor_copy(
                                VP[st][:, :, 0:DH],
                                pp[:].rearrange("p (h d) -> p h d", h=H_LOC))
                            nc.vector.memset(VP[st][:, :, DH:DH + 1], 1.0)

            # ---- phase C: attention ----
            with tc.tile_pool(name="epool", bufs=6) as epool, \
                 tc.tile_pool(name="norm", bufs=4) as norm, \
                 tc.tile_pool(name="psS", bufs=2, space="PSUM") as psS, \
                 tc.tile_pool(name="psY", bufs=2, space="PSUM") as psY:
                for g in range(4):          # head pair: local heads 2g, 2g+1
                    for qt in range(N_QT):  # q-chunk of 512
                        n_kt = 4 * (qt + 1)
                        q0, q1 = qt * 512, (qt + 1) * 512
                        yy = [psY.tile([DH + 1, 512], F32, tag=f"y{hh}", name=f"y{hh}")
                              for hh in range(2)]
                        for kp in range(n_kt // 2):
                            kts = (2 * kp, 2 * kp + 1)
                            ps2 = [psS.tile([128, 1024], F32, tag="s", name="s")
                                   for _ in range(2)]
                            # scores: group by head so consecutive matmuls
                            # stay on one PSUM tile (no bank cycling)
                            for hh in range(2):
                                rows = slice(hh * 64, hh * 64 + 64)
                                for j, kt in enumerate(kts):
                                    k0, k1 = kt * 128, (kt + 1) * 128
                                    nc.tensor.matmul(
                                        ps2[hh][:, j * 512:(j + 1) * 512],
                                        KT[g][rows, k0:k1],
                                        QT[g][rows, q0:q1],
                                        start=True, stop=True)
                            ee = [epool.tile([128, 1024], BF16, tag="e", name="e")
                                  for _ in range(2)]
                            diag = (kts[0] >= 4 * qt)
                            for hh in range(2):
                                if not diag:
                                    nc.scalar.activation(
                                        ee[hh][:], ps2[hh][:], EXP,
                                        scale=SCALE)
                                else:
                                    for j, kt in enumerate(kts):
                                        off = kt * 128 - qt * 512
                                        cb = j * 512
                                        if off > 0:
                                            nc.gpsimd.memset(
                                                ee[hh][:, cb:cb + off], 0.0)
                                        nc.scalar.activation(
                                            ee[hh][:, cb + off:cb + 512],
                                            ps2[hh][:, cb + off:cb + 512],
                                            EXP, scale=SCALE)
                                        nc.vector.tensor_mul(
                                            ee[hh][:, cb + off:cb + off + 128],
                                            ee[hh][:, cb + off:cb + off + 128],
                                            tri[:])
                            for hh in range(2):
                                for j, kt in enumerate(kts):
                                    nc.tensor.matmul(
                                        yy[hh][:],
                                        VP[kt][:, 2 * g + hh, :],
                                        ee[hh][:, j * 512:(j + 1) * 512],
                                        start=(kt == 0), stop=(kt == n_kt - 1))
                        for hh in range(2):
                            den = norm.tile([1, 512], F32, tag="den")
                            nc.vector.tensor_copy(den[:], yy[hh][DH:DH + 1, :])
                            rd = norm.tile([1, 512], F32, tag="rd")
                            nc.vector.reciprocal_approx_fast(rd[:], den[:])
                            rdb = norm.tile([DH, 512], F32, tag="rdb")
                            nc.gpsimd.partition_broadcast(rdb[:], rd[:])
                            yn = norm.tile([DH, 512], F32, tag="yn")
                            nc.vector.tensor_mul(yn[:], yy[hh][0:DH, :], rdb[:])
                            nc.sync.dma_start(
                                out[2 * g + hh, :, q0:q1], yn[:])
    nc.compile()
    return nc


def get_nc():
    if "nc" not in _NC_CACHE:
        _NC_CACHE["nc"] = build_nc()
    return _NC_CACHE["nc"]


def make_in_maps(x, W_q, W_k, W_v):
    in_maps = []
    for core in range(N_CORES):
        b, hg = core // 2, core % 2
        rows = slice(hg * D_LOC, (hg + 1) * D_LOC)
        bf = ml_dtypes.bfloat16
        in_maps.append({
            "xtd": np.ascontiguousarray(np.asarray(x[b], dtype=np.float32).T.astype(bf)),
            "wqt": np.ascontiguousarray(np.asarray(W_q[rows], dtype=np.float32).T.astype(bf)),
            "wkt": np.ascontiguousarray(np.asarray(W_k[rows], dtype=np.float32).T.astype(bf)),
            "wvt": np.ascontiguousarray(np.asarray(W_v[rows], dtype=np.float32).T.astype(bf)),
        })
    return in_maps


def assemble(results):
    Y = np.empty((B, H, S, DH), dtype=np.float32)
    for core in range(N_CORES):
        b, hg = core // 2, core % 2
        yc = results[core]["out"]  # [H_LOC, DH, S]
        Y[b, hg * H_LOC:(hg + 1) * H_LOC] = yc.transpose(0, 2, 1)
    return Y


def kernel(x, W_q, W_k, W_v):
    nc = get_nc()
    in_maps = make_in_maps(x, W_q, W_k, W_v)
    res = run_bass_kernel_spmd(nc, in_maps, list(range(N_CORES)))
    return assemble(res.results)



# revision 3
# speedup vs baseline: 1.1591x; 1.1591x over previous
"""Multihead causal attention on 8 TRN2 NeuronCores.

Sharding: core = (batch b, head-group hg): b = core//2, hg = core%2.
Each core gets x[b] (full sequence, [2048, 1024]) plus the weight rows for
its 8 heads (W[hg*512:(hg+1)*512, :]), computes Q/K/V projections and
causal attention for those (batch, head) pairs, and writes Y transposed
as [8, 64, 2048] (head, dh, seq); the host transposes back on gather.

v2 pipeline design (software-pipelined projections + attention):
  - Head pairs g=0..3. Attention for pair g runs interleaved with the
    Q/K projections for pair g+1 (and the V projection, during g=0), so
    ScalarE exp work overlaps projection matmuls instead of idling
    through a separate projection phase.
  - Scores in transposed layout scoresT[k, q] per head; the two heads of
    a pair run as concurrent 64-row PE tiles (tile_position auto-derived
    from base_partition 0/64).
  - Causal masking with no memsets: matmuls / exp activations are
    column-restricted to at-or-below-diagonal ranges, the triangular
    128x128 boundary blocks get one DVE multiply each, and the PV matmul
    reads only valid columns.
  - PV: yy[65, 512] per (head, q-chunk); V carries an appended ones
    column so row 64 accumulates the softmax denominator.  PV for kt
    pair p is emitted after the scores of pair p+1 (one-stage software
    pipeline) so TensorE never waits on the exp.
  - PE warm-up: dummy matmuls at T=0 (during input DMA) flip the HAM
    clock gate to 8/8 before real work arrives.
"""
import numpy as np
import ml_dtypes

import concourse.bass as bass
import concourse.tile as tile
from concourse import bacc, mybir
from concourse.bass_utils import run_bass_kernel_spmd

F32 = mybir.dt.float32
BF16 = mybir.dt.bfloat16
EXP = mybir.ActivationFunctionType.Exp

B, S, D, H, DH = 4, 2048, 1024, 16, 64
N_CORES = 8
H_LOC = 8           # heads per core
D_LOC = H_LOC * DH  # 512: projection output dim per core
N_CT = D // 128     # 8 contraction tiles
N_QT = S // 512     # 4 q-chunks of 512
SCALE = 1.0 / np.sqrt(DH)

_NC_CACHE = {}


def build_nc():
    nc = bacc.Bacc("TRN2", target_bir_lowering=False, debug=False,
                   num_devices=N_CORES)
    xtd = nc.dram_tensor("xtd", [D, S], BF16, kind="ExternalInput").ap()
    wqt = nc.dram_tensor("wqt", [D, D_LOC], BF16, kind="ExternalInput").ap()
    wkt = nc.dram_tensor("wkt", [D, D_LOC], BF16, kind="ExternalInput").ap()
    wvt = nc.dram_tensor("wvt", [D, D_LOC], BF16, kind="ExternalInput").ap()
    out = nc.dram_tensor("out", [H_LOC, DH, S], F32, kind="ExternalOutput").ap()

    # tri[kk, qq] = 1 iff qq >= kk (valid: query position >= key position)
    tri_np = (np.arange(128)[None, :] >= np.arange(128)[:, None])
    tri_dram = nc.inline_tensor(tri_np.astype(ml_dtypes.bfloat16), name="tri")

    with tile.TileContext(nc) as tc:
        with tc.tile_pool(name="consts", bufs=1) as consts, \
             tc.tile_pool(name="pers", bufs=1) as pers, \
             tc.tile_pool(name="ee", bufs=3) as epool, \
             tc.tile_pool(name="norm", bufs=2) as norm, \
             tc.tile_pool(name="psP", bufs=2, space="PSUM") as psP, \
             tc.tile_pool(name="psS", bufs=1, space="PSUM") as psS, \
             tc.tile_pool(name="psY", bufs=1, space="PSUM") as psY:

            tri = consts.tile([128, 128], BF16)
            nc.sync.dma_start(tri[:], tri_dram.ap())
            warm = consts.tile([128, 64], BF16, name="warm")
            nc.gpsimd.memset(warm[:], 0.0)

            # ---- PE warm-up: keep TensorE busy during input DMA ----
            for i in range(48):
                wp = psP.tile([128, 512], F32, tag="pp", name="wp")
                nc.tensor.matmul(wp[0:64, 0:64], warm[:], warm[:],
                                 start=True, stop=True)

            # ---- persistent SBUF tensors ----
            # x.T in two column halves per 128-row block (finer DMA deps)
            xTh = [[pers.tile([128, 1024], BF16, tag=f"xT{h}_{i}",
                              name=f"xT{h}_{i}") for i in range(N_CT)]
                   for h in range(2)]
            WT = {w: [pers.tile([128, D_LOC], BF16, tag=f"W{w}{i}",
                                name=f"W{w}{i}") for i in range(N_CT)]
                  for w in "qkv"}
            QT = [pers.tile([128, S], BF16, tag=f"QT{i}", name=f"QT{i}")
                  for i in range(4)]
            KT = [pers.tile([128, S], BF16, tag=f"KT{i}", name=f"KT{i}")
                  for i in range(4)]
            VP = [pers.tile([128, H_LOC, DH + 1], BF16, tag=f"VP{i}",
                            name=f"VP{i}") for i in range(16)]

            # ---- input DMA, ordered for earliest compute start ----
            for ct in range(N_CT):
                nc.sync.dma_start(WT["v"][ct][:], wvt[ct * 128:(ct + 1) * 128, :])
            for ct in range(N_CT):
                nc.sync.dma_start(xTh[0][ct][:], xtd[ct * 128:(ct + 1) * 128, 0:1024])
            for ct in range(N_CT):
                nc.sync.dma_start(WT["q"][ct][:], wqt[ct * 128:(ct + 1) * 128, :])
            for ct in range(N_CT):
                nc.sync.dma_start(WT["k"][ct][:], wkt[ct * 128:(ct + 1) * 128, :])
            for ct in range(N_CT):
                nc.sync.dma_start(xTh[1][ct][:], xtd[ct * 128:(ct + 1) * 128, 1024:2048])

            def xslice(c0, c1, ct):
                """x.T[ct*128:(ct+1)*128, c0:c1] from the half tiles."""
                h = c0 // 1024
                assert c1 <= (h + 1) * 1024
                return xTh[h][ct][:, c0 - h * 1024:c1 - h * 1024]

            # ---- projection bursts (8 accumulating matmuls + 1 cast) ----
            def qk_burst(w, g, qc):
                dst = QT if w == "q" else KT
                pp = psP.tile([128, 512], F32, tag="pp", name="pp")
                for ct in range(N_CT):
                    nc.tensor.matmul(
                        pp[:],
                        WT[w][ct][:, g * 128:(g + 1) * 128],
                        xslice(qc * 512, (qc + 1) * 512, ct),
                        start=(ct == 0), stop=(ct == N_CT - 1))
                nc.vector.tensor_copy(dst[g][:, qc * 512:(qc + 1) * 512], pp[:])

            def v_burst(st):
                pp = psP.tile([128, 512], F32, tag="pp", name="pp")
                for ct in range(N_CT):
                    nc.tensor.matmul(
                        pp[:],
                        xslice(st * 128, (st + 1) * 128, ct),
                        WT["v"][ct][:],
                        start=(ct == 0), stop=(ct == N_CT - 1))
                nc.vector.tensor_copy(
                    VP[st][:, :, 0:DH],
                    pp[:].rearrange("p (h d) -> p h d", h=H_LOC))
                nc.vector.memset(VP[st][:, :, DH:DH + 1], 1.0)

            # boundary fillers: fillers[g][qt] emitted before (g, qt)'s
            # score loop. qt=0 fillers run before pair g's attention.
            fillers = [[[] for _ in range(N_QT)] for _ in range(4)]
            fillers[0][0] = (
                [lambda st=st: v_burst(st) for st in range(4)]
                + [lambda: qk_burst("q", 0, 0), lambda: qk_burst("k", 0, 0)])
            for qt in range(1, 4):
                fillers[0][qt] = (
                    [lambda w="q", qc=qt: qk_burst(w, 0, qc),
                     lambda w="k", qc=qt: qk_burst(w, 0, qc)]
                    + [lambda st=st: v_burst(st)
                       for st in range(4 * qt, 4 * qt + 4)])
            # Q/K for pair g+1: qc emitted at (g, qt=qc+1), with qc=3 at
            # (g+1, qt=0) — always at least one full qt-stage early.
            for g in range(3):
                for qc in range(3):
                    fillers[g][qc + 1] += [
                        lambda w="q", gg=g + 1, qc=qc: qk_burst(w, gg, qc),
                        lambda w="k", gg=g + 1, qc=qc: qk_burst(w, gg, qc)]
                fillers[g + 1][0] += [
                    lambda w="q", gg=g + 1: qk_burst(w, gg, 3),
                    lambda w="k", gg=g + 1: qk_burst(w, gg, 3)]

            # ---- attention, software-pipelined over kt pairs ----
            def emit_scores(g, qt, kp):
                """Scores + exp + tri-mask for kt pair (2kp, 2kp+1).
                Returns the ee tiles (bf16 SBUF, [128, 1024])."""
                ps = [psS.tile([128, 1024], F32, tag=f"s{hh}", name=f"s{hh}")
                      for hh in range(2)]
                ee = [epool.tile([128, 1024], BF16, tag=f"e{hh}",
                                 name=f"e{hh}") for hh in range(2)]
                offs = [max(0, (2 * kp + j) * 128 - qt * 512) for j in (0, 1)]
                q0 = qt * 512
                # scores: j-outer, hh-inner so consecutive LDWEIGHTS
                # alternate 64-row PE tiles and overlap the other matmul
                for j in (0, 1):
                    kt = 2 * kp + j
                    for hh in range(2):
                        rows = slice(hh * 64, hh * 64 + 64)
                        nc.tensor.matmul(
                            ps[hh][:, j * 512 + offs[j]:(j + 1) * 512],
                            KT[g][rows, kt * 128:(kt + 1) * 128],
                            QT[g][rows, q0 + offs[j]:q0 + 512],
                            start=True, stop=True)
                diag = (2 * kp >= 4 * qt)
                for hh in range(2):
                    if not diag:
                        nc.scalar.activation(ee[hh][:], ps[hh][:], EXP,
                                             scale=SCALE)
                    else:
                        for j in (0, 1):
                            c0 = j * 512 + offs[j]
                            nc.scalar.activation(
                                ee[hh][:, c0:(j + 1) * 512],
                                ps[hh][:, c0:(j + 1) * 512], EXP, scale=SCALE)
                if diag:
                    for hh in range(2):
                        for j in (0, 1):
                            c0 = j * 512 + offs[j]
                            nc.vector.tensor_mul(
                                ee[hh][:, c0:c0 + 128],
                                ee[hh][:, c0:c0 + 128], tri[:])
                return ee

            def emit_pv(g, qt, kp, ee, yy, n_kt):
                for hh in range(2):
                    for j in (0, 1):
                        kt = 2 * kp + j
                        off = max(0, kt * 128 - qt * 512)
                        nc.tensor.matmul(
                            yy[hh][:, off:512],
                            VP[kt][:, 2 * g + hh, :],
                            ee[hh][:, j * 512 + off:(j + 1) * 512],
                            start=(kt == 0), stop=(kt == n_kt - 1))

            for g in range(4):
                for qt in range(N_QT):
                    for f in fillers[g][qt]:
                        f()
                    n_kt = 4 * (qt + 1)
                    q0 = qt * 512
                    yy = [psY.tile([DH + 1, 512], F32, tag=f"y{hh}",
                                   name=f"y{hh}") for hh in range(2)]
                    pend = None  # (kp, ee) awaiting PV emission
                    for kp in range(n_kt // 2):
                        ee = emit_scores(g, qt, kp)
                        if pend is not None:
                            emit_pv(g, qt, pend[0], pend[1], yy, n_kt)
                        pend = (kp, ee)
                    emit_pv(g, qt, pend[0], pend[1], yy, n_kt)
                    for hh in range(2):
                        den = norm.tile([1, 512], F32, tag="den")
                        nc.vector.tensor_copy(den[:], yy[hh][DH:DH + 1, :])
                        rd = norm.tile([1, 512], F32, tag="rd")
                        nc.vector.reciprocal_approx_fast(rd[:], den[:])
                        rdb = norm.tile([DH, 512], F32, tag="rdb")
                        nc.gpsimd.partition_broadcast(rdb[:], rd[:])
                        yn = norm.tile([DH, 512], F32, tag="yn")
                        nc.vector.tensor_mul(yn[:], yy[hh][0:DH, :], rdb[:])
                        nc.sync.dma_start(out[2 * g + hh, :, q0:q0 + 512],
                                          yn[:])
    nc.compile()
    return nc


def get_nc():
    if "nc" not in _NC_CACHE:
        _NC_CACHE["nc"] = build_nc()
    return _NC_CACHE["nc"]


def make_in_maps(x, W_q, W_k, W_v):
    in_maps = []
    for core in range(N_CORES):
        b, hg = core // 2, core % 2
        rows = slice(hg * D_LOC, (hg + 1) * D_LOC)
        bf = ml_dtypes.bfloat16
        in_maps.append({
            "xtd": np.ascontiguousarray(np.asarray(x[b], dtype=np.float32).T.astype(bf)),
            "wqt": np.ascontiguousarray(np.asarray(W_q[rows], dtype=np.float32).T.astype(bf)),
            "wkt": np.ascontiguousarray(np.asarray(W_k[rows], dtype=np.float32).T.astype(bf)),
            "wvt": np.ascontiguousarray(np.asarray(W_v[rows], dtype=np.float32).T.astype(bf)),
        })
    return in_maps


def assemble(results):
    Y = np.empty((B, H, S, DH), dtype=np.float32)
    for core in range(N_CORES):
        b, hg = core // 2, core % 2
        yc = results[core]["out"]  # [H_LOC, DH, S]
        Y[b, hg * H_LOC:(hg + 1) * H_LOC] = yc.transpose(0, 2, 1)
    return Y


def kernel(x, W_q, W_k, W_v):
    nc = get_nc()
    in_maps = make_in_maps(x, W_q, W_k, W_v)
    res = run_bass_kernel_spmd(nc, in_maps, list(range(N_CORES)))
    return assemble(res.results)


# revision 6
# speedup vs baseline: 1.2578x; 1.0852x over previous
"""Multihead causal attention on 8 TRN2 NeuronCores.

Sharding: core = (batch b, head-group hg): b = core//2, hg = core%2.
Each core gets x[b] (full sequence, [2048, 1024]) plus the weight rows for
its 8 heads (W[hg*512:(hg+1)*512, :]), computes Q/K/V projections and
causal attention for those (batch, head) pairs, and writes Y transposed
as [8, 64, 2048] (head, dh, seq); the host transposes back on gather.

v2 pipeline design (software-pipelined projections + attention):
  - Head pairs g=0..3. Attention for pair g runs interleaved with the
    Q/K projections for pair g+1 (and the V projection, during g=0), so
    ScalarE exp work overlaps projection matmuls instead of idling
    through a separate projection phase.
  - Scores in transposed layout scoresT[k, q] per head; the two heads of
    a pair run as concurrent 64-row PE tiles (tile_position auto-derived
    from base_partition 0/64).
  - Causal masking with no memsets: matmuls / exp activations are
    column-restricted to at-or-below-diagonal ranges, the triangular
    128x128 boundary blocks get one DVE multiply each, and the PV matmul
    reads only valid columns.
  - PV: yy[65, 512] per (head, q-chunk); V carries an appended ones
    column so row 64 accumulates the softmax denominator.  PV for kt
    pair p is emitted after the scores of pair p+1 (one-stage software
    pipeline) so TensorE never waits on the exp.
  - PE warm-up: dummy matmuls at T=0 (during input DMA) flip the HAM
    clock gate to 8/8 before real work arrives.
"""
import numpy as np
import ml_dtypes

import concourse.bass as bass
import concourse.tile as tile
from concourse import bacc, mybir
from concourse.bass_utils import run_bass_kernel_spmd

F32 = mybir.dt.float32
BF16 = mybir.dt.bfloat16
EXP = mybir.ActivationFunctionType.Exp

B, S, D, H, DH = 4, 2048, 1024, 16, 64
N_CORES = 8
H_LOC = 8           # heads per core
D_LOC = H_LOC * DH  # 512: projection output dim per core
N_CT = D // 128     # 8 contraction tiles
N_QT = S // 512     # 4 q-chunks of 512
SCALE = 1.0 / np.sqrt(DH)

_NC_CACHE = {}


def build_nc():
    nc = bacc.Bacc("TRN2", target_bir_lowering=False, debug=False,
                   num_devices=N_CORES)
    xtd = nc.dram_tensor("xtd", [D, S], BF16, kind="ExternalInput").ap()
    wqt = nc.dram_tensor("wqt", [D, D_LOC], BF16, kind="ExternalInput").ap()
    wkt = nc.dram_tensor("wkt", [D, D_LOC], BF16, kind="ExternalInput").ap()
    wvt = nc.dram_tensor("wvt", [D, D_LOC], BF16, kind="ExternalInput").ap()
    out = nc.dram_tensor("out", [H_LOC, DH, S], F32, kind="ExternalOutput").ap()

    # tri[kk, qq] = 1 iff qq >= kk (valid: query position >= key position)
    tri_np = (np.arange(128)[None, :] >= np.arange(128)[:, None])
    tri_dram = nc.inline_tensor(tri_np.astype(ml_dtypes.bfloat16), name="tri")

    with tile.TileContext(nc) as tc:
        with tc.tile_pool(name="consts", bufs=1) as consts, \
             tc.tile_pool(name="pers", bufs=1) as pers, \
             tc.tile_pool(name="ee", bufs=3) as epool, \
             tc.tile_pool(name="norm", bufs=2) as norm, \
             tc.tile_pool(name="psP", bufs=2, space="PSUM") as psP, \
             tc.tile_pool(name="psS", bufs=1, space="PSUM") as psS, \
             tc.tile_pool(name="psY", bufs=1, space="PSUM") as psY:

            tri = consts.tile([128, 128], BF16)
            nc.sync.dma_start(tri[:], tri_dram.ap())
            warm = consts.tile([128, 128], BF16, name="warm")
            nc.gpsimd.memset(warm[:], 0.0)

            # ---- PE warm-up: ~8.6us of cold-rate matmuls flips the HAM
            # clock gate to 8/8 while the input DMA is still in flight ----
            for i in range(80):
                wp = psP.tile([128, 512], F32, tag="pp", name="wp")
                nc.tensor.matmul(wp[:, 0:128], warm[:], warm[:],
                                 start=True, stop=True)

            # ---- persistent SBUF tensors ----
            # x.T in two column halves per 128-row block (finer DMA deps)
            xTh = [[pers.tile([128, 1024], BF16, tag=f"xT{h}_{i}",
                              name=f"xT{h}_{i}") for i in range(N_CT)]
                   for h in range(2)]
            WT = {w: [pers.tile([128, D_LOC], BF16, tag=f"W{w}{i}",
                                name=f"W{w}{i}") for i in range(N_CT)]
                  for w in "qkv"}
            QT = [pers.tile([128, S], BF16, tag=f"QT{i}", name=f"QT{i}")
                  for i in range(4)]
            KT = [pers.tile([128, S], BF16, tag=f"KT{i}", name=f"KT{i}")
                  for i in range(4)]
            VP = [pers.tile([128, H_LOC, DH + 1], BF16, tag=f"VP{i}",
                            name=f"VP{i}") for i in range(16)]

            # ---- input DMA, ordered for earliest compute start ----
            for ct in range(N_CT):
                nc.sync.dma_start(WT["v"][ct][:], wvt[ct * 128:(ct + 1) * 128, :])
            for ct in range(N_CT):
                nc.sync.dma_start(xTh[0][ct][:], xtd[ct * 128:(ct + 1) * 128, 0:1024])
            for ct in range(N_CT):
                nc.sync.dma_start(WT["q"][ct][:], wqt[ct * 128:(ct + 1) * 128, :])
            for ct in range(N_CT):
                nc.sync.dma_start(WT["k"][ct][:], wkt[ct * 128:(ct + 1) * 128, :])
            for ct in range(N_CT):
                nc.sync.dma_start(xTh[1][ct][:], xtd[ct * 128:(ct + 1) * 128, 1024:2048])

            def xslice(c0, c1, ct):
                """x.T[ct*128:(ct+1)*128, c0:c1] from the half tiles."""
                h = c0 // 1024
                assert c1 <= (h + 1) * 1024
                return xTh[h][ct][:, c0 - h * 1024:c1 - h * 1024]

            # ---- projection bursts (8 accumulating matmuls + 1 cast) ----
            def qk_burst(w, g, qc):
                dst = QT if w == "q" else KT
                pp = psP.tile([128, 512], F32, tag="pp", name="pp")
                for ct in range(N_CT):
                    nc.tensor.matmul(
                        pp[:],
                        WT[w][ct][:, g * 128:(g + 1) * 128],
                        xslice(qc * 512, (qc + 1) * 512, ct),
                        start=(ct == 0), stop=(ct == N_CT - 1))
                nc.vector.tensor_copy(dst[g][:, qc * 512:(qc + 1) * 512], pp[:])

            def v_burst(st):
                pp = psP.tile([128, 512], F32, tag="pp", name="pp")
                for ct in range(N_CT):
                    nc.tensor.matmul(
                        pp[:],
                        xslice(st * 128, (st + 1) * 128, ct),
                        WT["v"][ct][:],
                        start=(ct == 0), stop=(ct == N_CT - 1))
                nc.vector.tensor_copy(
                    VP[st][:, :, 0:DH],
                    pp[:].rearrange("p (h d) -> p h d", h=H_LOC))
                nc.vector.memset(VP[st][:, :, DH:DH + 1], 1.0)

            # JIT fillers: fillers[g][qt] is projection work interleaved
            # INTO stage (g, qt)'s kt-pair loop, keeping TensorE dense so
            # the HAM clock gate never re-throttles.  Q/K for q-chunk qc
            # of pair g is produced during stage (g, qc-1) — one stage
            # ahead of first use — and qc=0 during stage (g-1, qt=3).
            fillers = [[[] for _ in range(N_QT)] for _ in range(4)]
            for g in range(4):
                for qt in range(N_QT):
                    fl = fillers[g][qt]
                    if g == 0 and qt < 3:  # V blocks for stage (0, qt+1)
                        fl += [lambda st=st: v_burst(st)
                               for st in range(4 * qt + 4, 4 * qt + 8)]
                    if qt < 3:
                        fl += [lambda w="q", gg=g, qc=qt + 1: qk_burst(w, gg, qc),
                               lambda w="k", gg=g, qc=qt + 1: qk_burst(w, gg, qc)]
                    elif g < 3:
                        fl += [lambda w="q", gg=g + 1: qk_burst(w, gg, 0),
                               lambda w="k", gg=g + 1: qk_burst(w, gg, 0)]

            # ---- attention, software-pipelined over kt pairs ----
            def emit_scores(g, qt, kp):
                """Scores + exp + tri-mask for kt pair (2kp, 2kp+1).
                Returns the ee tiles (bf16 SBUF, [128, 1024])."""
                ps = [psS.tile([128, 1024], F32, tag=f"s{hh}", name=f"s{hh}")
                      for hh in range(2)]
                ee = [epool.tile([128, 1024], BF16, tag=f"e{hh}",
                                 name=f"e{hh}") for hh in range(2)]
                offs = [max(0, (2 * kp + j) * 128 - qt * 512) for j in (0, 1)]
                q0 = qt * 512
                # scores: j-outer, hh-inner so consecutive LDWEIGHTS
                # alternate 64-row PE tiles and overlap the other matmul
                for j in (0, 1):
                    kt = 2 * kp + j
                    for hh in range(2):
                        rows = slice(hh * 64, hh * 64 + 64)
                        nc.tensor.matmul(
                            ps[hh][:, j * 512 + offs[j]:(j + 1) * 512],
                            KT[g][rows, kt * 128:(kt + 1) * 128],
                            QT[g][rows, q0 + offs[j]:q0 + 512],
                            start=True, stop=True)
                diag = (2 * kp >= 4 * qt)
                for hh in range(2):
                    if not diag:
                        nc.scalar.activation(ee[hh][:], ps[hh][:], EXP,
                                             scale=SCALE)
                    else:
                        for j in (0, 1):
                            c0 = j * 512 + offs[j]
                            nc.scalar.activation(
                                ee[hh][:, c0:(j + 1) * 512],
                                ps[hh][:, c0:(j + 1) * 512], EXP, scale=SCALE)
                if diag:
                    for hh in range(2):
                        for j in (0, 1):
                            c0 = j * 512 + offs[j]
                            nc.vector.tensor_mul(
                                ee[hh][:, c0:c0 + 128],
                                ee[hh][:, c0:c0 + 128], tri[:])
                return ee

            def emit_pv(g, qt, kp, ee, yy, n_kt):
                for hh in range(2):
                    for j in (0, 1):
                        kt = 2 * kp + j
                        off = max(0, kt * 128 - qt * 512)
                        nc.tensor.matmul(
                            yy[hh][:, off:512],
                            VP[kt][:, 2 * g + hh, :],
                            ee[hh][:, j * 512 + off:(j + 1) * 512],
                            start=(kt == 0), stop=(kt == n_kt - 1))

            # startup projections: V[0:4] + Q/K q-chunk 0 for pair 0
            for st in range(4):
                v_burst(st)
            qk_burst("q", 0, 0)
            qk_burst("k", 0, 0)

            for g in range(4):
                for qt in range(N_QT):
                    fl = fillers[g][qt]
                    n_kt = 4 * (qt + 1)
                    n_kp = n_kt // 2
                    q0 = qt * 512
                    yy = [psY.tile([DH + 1, 512], F32, tag=f"y{hh}",
                                   name=f"y{hh}") for hh in range(2)]
                    pend = None  # (kp, ee) awaiting PV emission
                    fi = 0  # fillers emitted so far
                    for kp in range(n_kp):
                        ee = emit_scores(g, qt, kp)
                        if pend is not None:
                            emit_pv(g, qt, pend[0], pend[1], yy, n_kt)
                        pend = (kp, ee)
                        # interleave a proportional share of the fillers
                        want = (len(fl) * (kp + 1)) // n_kp
                        while fi < want:
                            fl[fi]()
                            fi += 1
                    while fi < len(fl):
                        fl[fi]()
                        fi += 1
                    emit_pv(g, qt, pend[0], pend[1], yy, n_kt)
                    for hh in range(2):
                        den = norm.tile([1, 512], F32, tag="den")
                        nc.vector.tensor_copy(den[:], yy[hh][DH:DH + 1, :])
                        rd = norm.tile([1, 512], F32, tag="rd")
                        nc.vector.reciprocal_approx_fast(rd[:], den[:])
                        rdb = norm.tile([DH, 512], F32, tag="rdb")
                        nc.gpsimd.partition_broadcast(rdb[:], rd[:])
                        yn = norm.tile([DH, 512], F32, tag="yn")
                        nc.vector.tensor_mul(yn[:], yy[hh][0:DH, :], rdb[:])
                        nc.sync.dma_start(out[2 * g + hh, :, q0:q0 + 512],
                                          yn[:])
    nc.compile()
    return nc


def get_nc():
    if "nc" not in _NC_CACHE:
        _NC_CACHE["nc"] = build_nc()
    return _NC_CACHE["nc"]


def make_in_maps(x, W_q, W_k, W_v):
    in_maps = []
    for core in range(N_CORES):
        b, hg = core // 2, core % 2
        rows = slice(hg * D_LOC, (hg + 1) * D_LOC)
        bf = ml_dtypes.bfloat16
        in_maps.append({
            "xtd": np.ascontiguousarray(np.asarray(x[b], dtype=np.float32).T.astype(bf)),
            "wqt": np.ascontiguousarray(np.asarray(W_q[rows], dtype=np.float32).T.astype(bf)),
            "wkt": np.ascontiguousarray(np.asarray(W_k[rows], dtype=np.float32).T.astype(bf)),
            "wvt": np.ascontiguousarray(np.asarray(W_v[rows], dtype=np.float32).T.astype(bf)),
        })
    return in_maps


def assemble(results):
    Y = np.empty((B, H, S, DH), dtype=np.float32)
    for core in range(N_CORES):
        b, hg = core // 2, core % 2
        yc = results[core]["out"]  # [H_LOC, DH, S]
        Y[b, hg * H_LOC:(hg + 1) * H_LOC] = yc.transpose(0, 2, 1)
    return Y


def kernel(x, W_q, W_k, W_v):
    nc = get_nc()
    in_maps = make_in_maps(x, W_q, W_k, W_v)
    res = run_bass_kernel_spmd(nc, in_maps, list(range(N_CORES)))
    return assemble(res.results)
